# revision 25
# baseline (speedup 1.0000x reference)
"""Trainium2 Bass kernel for nn_DinoGazeSpade (segment_reduce + SPADE stack).

Layout: 8 cores; image k = core//2; each core computes rows [16h, 16h+16) of
the 32x32 grid (h = core%2) for the heavy layer-0 convs AND its own half of
out0 = softplus(LN0-linearized layer 0), then the light tail (layers 1/2) is
computed redundantly on the FULL image by both cores of a pair so the
LN1/LN2 statistics need no further collective.

Cross-core traffic is two pairwise AllGathers (plus one dummy warmup that
absorbs the ~45+25us ncfw boot):
  - "early" (~84KB): LN0 partial stats + the h-conv partials the full-image
    tail needs (C1 | G2 | C2 | G1), issued mid-kernel so it hides under
    conv_g; B1 = W1^T gp11(G1-full) is then computed right after conv_g.
  - "late" (~8.3KB): og1 = out0*gp11 for our own half (f16) and the LN1
    partial sums (S,Q) riding as a bitcast -- computed locally right after
    conv_g finishes (r0/nm0 are already known from the early exchange, so
    each core can softplus its own half without waiting); the full-image
    A1 = W1^T og1f is two small matmuls after the exchange.
All assembled full-image tensors are rank-ordered (row-half 0 cols 0:512,
half 1 cols 512:1024) so the SPMD program is identical on both cores; the
host reads the full 32x32 output from core 2k.

Key algebra vs a direct port:
  - sm (painted+resized semantic map) takes at most 64 distinct values per
    image, so conv3x3(sm, ws) collapses to T = sums^T @ ws followed by 9
    gather-matmuls against corner-count one-hots Gr (1/(4*cnt) folded in).
  - SPADE wb convs fold through the 1x1 convs on the host (wf = conv_w @ wb);
    B0 = W0^T gp1 folds to an 8-channel conv of h0 plus a constant.
  - LayerNorm linearized through the 1x1 convs: z = r*A + (-mu*r)*B + C.
  - softplus(z) = Ln(Exp(z) + 1) -- two ACT ops on the pinned
    natural_log_exp table set (z is bounded ~+-15 here, exp is safe).
  - LN sums ride ACT accum_out / DVE affine_mul_reduce; the cross-channel
    total + partition broadcast is one all-ones matmul; rsqrt via DVE
    Newton (no extra ACT tables).
"""
import os
import numpy as np
from contextlib import ExitStack

import concourse.bass as bass
import concourse.mybir as mybir
import concourse.tile as tile
from concourse import bacc
from concourse.bass_utils import run_bass_kernel_spmd
from concourse.masks import make_identity

f32 = mybir.dt.float32
f16 = mybir.dt.float16
i32 = mybir.dt.int32
AF = mybir.ActivationFunctionType
ALU = mybir.AluOpType

NSEG = 64
B, Cd, Hp, Wp, H, W, Cm, HID = 4, 384, 32, 32, 448, 448, 1536, 128
NPOS = Hp * Wp          # 1024
HROWS = 16              # rows per core
GRR = 20                # Gr rows incl 2-halo
HR = HROWS + 2          # h rows incl 1-halo = 18
W34 = 34                # padded width
PAIRS = [[0, 1], [2, 3], [4, 5], [6, 7]]
TAPS = [(t // 3, t % 3) for t in range(9)]
MAGIC = 0x5F3759DF

# early-exchange payload layout (f32 elements within d_ex_l)
EX_C1 = 0                 # C1                             [16, 512]
EX_G2 = 16 * 512          # G2                             [16, 512]
EX_C2 = 32 * 512          # C2                             [1, 512]
EX_G1 = 33 * 512          # G1                             [8, 512]
EX_ST = 41 * 512          # LN0 partial (sum, sumsq)       [2]
EX_N = EX_ST + 2
# late-exchange payload layout (f16): og1 = out0*gp11 own half, plus the
# LN1 per-channel partial sums riding as a bitcast of [8,2] f32
LT_OG = 0                 # og1 own half f16               [8, 512]
LT_SQ = 8 * 512           # LN1 partials, f32 bitcast      [32]
LT_N = LT_SQ + 32

LAST_RESULTS = None
_BUILT = None


def _newton_rsqrt(nc, pool, var, p, tag, n_iter=2):
    """r = 1/sqrt(var) for var [p,1] f32 > 0, DVE only (no ACT tables)."""
    ib = pool.tile([p, 1], i32, tag=f"rs_ib{tag}", name=f"rs_ib{tag}")
    nc.vector.tensor_scalar(out=ib, in0=var[:, 0:1].bitcast(i32), scalar1=1,
                            scalar2=-1, op0=ALU.logical_shift_right,
                            op1=ALU.bitwise_xor)
    nc.vector.tensor_scalar(out=ib, in0=ib, scalar1=MAGIC + 1, scalar2=None,
                            op0=ALU.add)
    y = ib[:, 0:1].bitcast(f32)
    vm = pool.tile([p, 1], f32, tag=f"rs_v{tag}", name=f"rs_v{tag}")
    nc.vector.tensor_scalar_mul(vm, var, -0.5)
    t = pool.tile([p, 1], f32, tag=f"rs_t{tag}", name=f"rs_t{tag}")
    r = pool.tile([p, 1], f32, tag=f"rs_r{tag}", name=f"rs_r{tag}")
    ja = pool.tile([p, 1], f32, tag=f"rs_j{tag}", name=f"rs_j{tag}")
    cur = y
    for it in range(n_iter):
        nc.vector.tensor_tensor(out=t, in0=cur, in1=cur, op=ALU.mult)
        nc.vector.affine_mul_reduce(out=r, accum_out=ja, in0=t, in1=cur,
                                    scale=vm[:, 0:1], bias=1.5)
        cur = r
    return r


def _stats_chain(nc, pool, stM, n_inst, p, tag, n_iter=2, pair=False):
    """stM [p,2] = (S, Q) total sums broadcast across p partitions (or
    [p,4] = per-half partials when pair=True). Returns (r, -mu*r) [p,1]."""
    if pair:
        sq = pool.tile([p, 2], f32, tag=f"wk_s{tag}", name=f"wk_s{tag}")
        nc.vector.tensor_tensor(out=sq, in0=stM[:, 0:2], in1=stM[:, 2:4],
                                op=ALU.add)
        stM = sq
    msq = pool.tile([p, 2], f32, tag=f"wk_m{tag}", name=f"wk_m{tag}")
    nc.vector.tensor_scalar_mul(msq, stM, 1.0 / n_inst)      # (mu, E[x^2])
    var = pool.tile([p, 1], f32, tag=f"wk_v{tag}", name=f"wk_v{tag}")
    nc.vector.tensor_tensor(out=var, in0=msq[:, 0:1], in1=msq[:, 0:1], op=ALU.mult)
    nc.vector.tensor_tensor(out=var, in0=msq[:, 1:2], in1=var, op=ALU.subtract)
    r = _newton_rsqrt(nc, pool, var, p, tag, n_iter)
    m = pool.tile([p, 1], f32, tag=f"wk_nm{tag}", name=f"wk_nm{tag}")
    nc.vector.tensor_tensor(out=m, in0=msq[:, 0:1], in1=r, op=ALU.mult)
    nc.vector.tensor_scalar_mul(m, m, -1.0)                  # -mu*r
    return r, m


def _build_nc():
    nc = bacc.Bacc("TRN2", num_devices=8)

    # ---------------- DRAM I/O ----------------
    d_x = nc.dram_tensor("x", [128, 12, 512], f16, kind="ExternalInput")
    d_ft = nc.dram_tensor("ft", [128, 8, 384], f16, kind="ExternalInput")
    d_ids = nc.dram_tensor("ids", [128, 8], f32, kind="ExternalInput")
    d_cid = nc.dram_tensor("cid", [128, 5, 4], f32, kind="ExternalInput")
    d_hmask = nc.dram_tensor("hmask", [HR], f16, kind="ExternalInput")
    d_wst = nc.dram_tensor("wst", [128, 3, 3456], f16, kind="ExternalInput")
    d_wg = nc.dram_tensor("wg", [128, 12, 9, 128], f16, kind="ExternalInput")
    d_w0t = nc.dram_tensor("w0t", [128, 12, 8], f16, kind="ExternalInput")
    d_wbc0 = nc.dram_tensor("wbc0", [128, 9, 16], f16, kind="ExternalInput")
    d_wgc1 = nc.dram_tensor("wgc1", [128, 9, 24], f16, kind="ExternalInput")
    d_wgc2 = nc.dram_tensor("wgc2", [128, 9, 17], f16, kind="ExternalInput")
    d_w12 = nc.dram_tensor("w12", [16, 17], f16, kind="ExternalInput")
    d_bs = nc.dram_tensor("bs", [128, 3], f32, kind="ExternalInput")
    d_gb0 = nc.dram_tensor("gb0", [128, 12], f32, kind="ExternalInput")
    # sc2 cols: gb2(16) | b1f(16) | gb1(8) | b0f(8) | b0c(8) | b2f(1)
    d_sc2 = nc.dram_tensor("sc2", [16, 6], f32, kind="ExternalInput")
    d_out = nc.dram_tensor("out_full", [1024], f32, kind="ExternalOutput")

    warm_l = nc.dram_tensor("warm_l", [2], f32)
    warm_g = nc.dram_tensor("warm_g", [4], f32)
    d_ex_l = nc.dram_tensor("ex_l", [EX_N], f32)
    d_ex_g = nc.dram_tensor("ex_g", [2, EX_N], f32)
    d_lt_l = nc.dram_tensor("lt_l", [LT_N], f16)
    d_lt_g = nc.dram_tensor("lt_g", [2, LT_N], f16)

    # pin the combined exp+ln ACT table once; all other ACT funcs used
    # (identity/copy/relu/square) are co-resident, so the table-load pass
    # finds every activation covered and inserts no further swaps.
    from concourse.hw_specs import get_activation_tables
    tabs = list(get_activation_tables(nc.m.arch))
    NLE_SET = tabs.index("natural_log_exp_and_others")
    nc.scalar.add_instruction(mybir.InstLoadActFuncSet(
        name=nc.get_next_instruction_name(), act_func_set_id=NLE_SET))

    with ExitStack() as ctx:
        tc = ctx.enter_context(tile.TileContext(nc, num_cores=8))
        cpool = ctx.enter_context(tc.tile_pool(name="consts", bufs=1))
        dpool = ctx.enter_context(tc.tile_pool(name="data", bufs=1))
        spool = ctx.enter_context(tc.tile_pool(name="small", bufs=1))
        ps = ctx.enter_context(tc.tile_pool(name="ps", bufs=1, space="PSUM"))

        def MAIN(shape, name, dt=f32):
            return ps.tile(shape, dt, tag="ps_main", bufs=3, name=name)

        def SMALL(shape, name):
            return ps.tile(shape, f32, tag="ps_small", bufs=2, name=name)

        def TAIL(shape, name):
            return ps.tile(shape, f32, tag="ps_tail", bufs=2, name=name)

        # ---- constants ----
        iot = cpool.tile([128, 64], f32)
        nc.gpsimd.iota(iot, pattern=[[1, 64]], base=0, channel_multiplier=0,
                       allow_small_or_imprecise_dtypes=True)
        ident = cpool.tile([128, 128], f16)
        make_identity(nc, ident)
        ones16 = cpool.tile([128, 1], f16)
        nc.gpsimd.memset(ones16, 1.0)
        ones32 = cpool.tile([128, 1], f32)
        nc.gpsimd.memset(ones32, 1.0)
        allon = cpool.tile([16, 16], f32)
        nc.gpsimd.memset(allon, 1.0)
        wrm = cpool.tile([128, 512], f16)
        nc.gpsimd.memset(wrm, 0.0)

        # --------- DMAs, ordered so early-needed data lands first ---------
        feats = dpool.tile([128, 8, 384], f16)
        nc.sync.dma_start(out=feats[:, 0:4, :], in_=d_ft[:, 0:4, :])
        idst = cpool.tile([128, 8], f32)
        nc.sync.dma_start(out=idst, in_=d_ids[:, :])
        nc.sync.dma_start(out=feats[:, 4:8, :], in_=d_ft[:, 4:8, :])
        cidt = cpool.tile([128, 5, 4], f32)
        nc.sync.dma_start(out=cidt, in_=d_cid[:, :, :])
        bs_t = cpool.tile([128, 3], f32)
        nc.sync.dma_start(out=bs_t, in_=d_bs[:, :])
        wst_t = cpool.tile([128, 3, 3456], f16)
        nc.sync.dma_start(out=wst_t[:, :, 0:1152], in_=d_wst[:, :, 0:1152])
        xt = dpool.tile([128, 12, 512], f16)
        nc.sync.dma_start(out=xt, in_=d_x[:, :, :])
        w0t_t = cpool.tile([128, 12, 8], f16)
        nc.sync.dma_start(out=w0t_t, in_=d_w0t[:, :, :])
        wbc0_t = cpool.tile([128, 9, 16], f16)
        nc.sync.dma_start(out=wbc0_t, in_=d_wbc0[:, :, :])
        wgc1_t = cpool.tile([128, 9, 24], f16)
        nc.sync.dma_start(out=wgc1_t, in_=d_wgc1[:, :, :])
        wgc2_t = cpool.tile([128, 9, 17], f16)
        nc.sync.dma_start(out=wgc2_t, in_=d_wgc2[:, :, :])
        gb0_t = cpool.tile([128, 12], f32)
        nc.sync.dma_start(out=gb0_t, in_=d_gb0[:, :])
        nc.sync.dma_start(out=wst_t[:, :, 1152:2304], in_=d_wst[:, :, 1152:2304])
        w12_t = cpool.tile([16, 17], f16)
        nc.sync.dma_start(out=w12_t, in_=d_w12[:, :])
        w1t_t = w12_t[0:8, 0:16]
        w2t_t = w12_t[0:16, 16:17]
        sc2_t = cpool.tile([16, 6], f32)
        nc.sync.dma_start(out=sc2_t, in_=d_sc2[:, :])
        gb2_t = sc2_t[:, 0:1]
        b1fb = sc2_t[:, 1:2]
        gb1_t = sc2_t[0:8, 2:3]
        b0fb = sc2_t[0:8, 3:4]
        b0cb = sc2_t[0:8, 4:5]
        b2fb = sc2_t[0:1, 5:6]
        nc.sync.dma_start(out=wst_t[:, :, 2304:3456], in_=d_wst[:, :, 2304:3456])
        hmask_bc = cpool.tile([128, HR], f16)
        nc.gpsimd.dma_start(out=hmask_bc, in_=d_hmask[None, :].to_broadcast([128, HR]))
        wg_t = cpool.tile([128, 12, 9, 128], f16)
        nc.sync.dma_start(out=wg_t[:, 0:4], in_=d_wg[:, 0:4])
        nc.sync.dma_start(out=wg_t[:, 4:8], in_=d_wg[:, 4:8])
        nc.sync.dma_start(out=wg_t[:, 8:12], in_=d_wg[:, 8:12])

        # ---- PE warmup: dummy matmuls while the first DMAs land, so the
        # HAM clock-gate is already at 8/8 when real work starts ----
        for wi in range(12):
            pw = MAIN([128, 512], f"warm{wi}")
            nc.tensor.matmul(pw, ident, wrm, start=True, stop=True)

        # ---------------- one-hots ----------------
        oh_t = dpool.tile([128, 8, 64], f16)
        for qc in range(8):
            nc.vector.tensor_scalar(out=oh_t[:, qc, :], in0=iot,
                                    scalar1=idst[:, qc:qc + 1], scalar2=None,
                                    op0=ALU.is_equal)

        # ---------------- segment sums^T [384, 64] + counts ----------------
        sumsT = dpool.tile([128, 3, 64], f16)
        for mc in range(3):
            pT = MAIN([128, 64], f"pT{mc}")
            for qc in range(8):
                nc.tensor.matmul(pT, feats[:, qc, mc * 128:(mc + 1) * 128],
                                 oh_t[:, qc, :], start=(qc == 0), stop=(qc == 7))
            nc.scalar.copy(sumsT[:, mc, :], pT)
        cntp = MAIN([64, 1], "cntp")
        for qc in range(8):
            nc.tensor.matmul(cntp, oh_t[:, qc, :], ones16,
                             start=(qc == 0), stop=(qc == 7))
        recip4 = spool.tile([64, 1], f32, tag="recip4")
        nc.vector.tensor_scalar(out=recip4, in0=cntp, scalar1=1.0,
                                scalar2=4.0, op0=ALU.max, op1=ALU.mult)
        nc.vector.reciprocal(out=recip4, in_=recip4)

        # ---------------- Gr corner-count masks [64, 20, 34] ----------------
        gacc = dpool.tile([128, 5, 64], f16)
        gtmp = dpool.tile([128, 64], f16)
        for jc in range(5):
            nc.vector.tensor_scalar(out=gacc[:, jc, :], in0=iot,
                                    scalar1=cidt[:, jc, 0:1], scalar2=None,
                                    op0=ALU.is_equal)
            for corner in range(1, 4):
                nc.vector.tensor_scalar(out=gtmp, in0=iot,
                                        scalar1=cidt[:, jc, corner:corner + 1],
                                        scalar2=None, op0=ALU.is_equal)
                nc.vector.tensor_tensor(out=gacc[:, jc, :], in0=gacc[:, jc, :],
                                        in1=gtmp, op=ALU.add)
        gr_pad = dpool.tile([64, GRR, W34], f16)
        nc.gpsimd.memset(gr_pad, 0.0)
        for jc in range(5):
            ptr = MAIN([64, 128], f"ptr{jc}", dt=f16)
            nc.tensor.transpose(ptr, gacc[:, jc, :], ident)
            nc.scalar.copy(gr_pad[:, jc * 4:(jc + 1) * 4, 1:33],
                           ptr.rearrange("p (r c) -> p r c", c=32))
        # fold the 1/(4*cnt) normalization into Gr rows (per-segment scalar)
        nc.vector.tensor_scalar_mul(gr_pad, gr_pad, recip4)

        # ---------------- T tables [64, 3456] = sums^T @ ws ----------------
        T_sb = dpool.tile([64, 3456], f16)

        def t_build(cv):
            base = cv * 1152
            for s0, ncols in [(0, 512), (512, 512), (1024, 128)]:
                psT = MAIN([64, 512], f"psT{cv}{s0}")
                for mc in range(3):
                    nc.tensor.matmul(psT[:, 0:ncols], sumsT[:, mc, :],
                                     wst_t[:, mc, base + s0:base + s0 + ncols],
                                     start=(mc == 0), stop=(mc == 2))
                nc.scalar.copy(T_sb[:, base + s0:base + s0 + ncols],
                               psT[:, 0:ncols])

        # ---------------- h gather-conv helper ----------------
        hp_tiles = []
        for cv in range(3):
            hp = dpool.tile([128, HR, W34], f16, tag=f"hpad{cv}", name=f"hpad{cv}")
            nc.gpsimd.memset(hp, 0.0)
            hp_tiles.append(hp)

        def h_conv(cv):
            hp = hp_tiles[cv]
            for half in range(2):
                psh = MAIN([128, 288], f"psh{cv}{half}")
                r0 = half * 9
                for t, (dy, dx) in enumerate(TAPS):
                    ti = (cv * 9 + t) * 128
                    nc.tensor.matmul(
                        psh, T_sb[:, ti:ti + 128],
                        gr_pad[:, r0 + dy:r0 + dy + 9, dx:dx + 32],
                        start=(t == 0), stop=(t == 8))
                nc.scalar.activation(
                    out=hp[:, r0:r0 + 9, 1:33],
                    in_=psh.rearrange("p (r c) -> p r c", c=32),
                    func=AF.Relu, bias=bs_t[:, cv:cv + 1])
            nc.vector.tensor_tensor(
                out=hp, in0=hp,
                in1=hmask_bc[:, :, None].to_broadcast([128, HR, W34]),
                op=ALU.mult)
            return hp

        t_build(0)
        h0p = h_conv(0)
        t_build(1)
        h1p = h_conv(1)
        t_build(2)
        h2p = h_conv(2)

        # ---------- small h-convs ----------
        def small_conv(wtab, ncols, hpx, name):
            p = SMALL([ncols, 512], name)
            for t, (dy, dx) in enumerate(TAPS):
                nc.tensor.matmul(p, wtab[:, t, 0:ncols],
                                 hpx[:, dy:dy + 16, dx:dx + 32],
                                 start=(t == 0), stop=(t == 8))
            return p

        # B0|C0 (local only: feeds u0 for our own half)
        psBC0 = small_conv(wbc0_t, 16, h0p, "psBC0")
        bc0sb = dpool.tile([16, 512], f32, name="bc0sb")
        nc.scalar.copy(bc0sb, psBC0)
        c0sb = dpool.tile([8, 512], f32, name="c0sb")
        nc.sync.dma_start(out=c0sb, in_=bc0sb[8:16, :])   # partition shift

        # G1|C1: G1 stays local (gp11), C1 goes in the early exchange
        psGC1 = small_conv(wgc1_t, 24, h1p, "psGC1")
        gc1sb = dpool.tile([24, 512], f32, name="gc1sb")
        nc.scalar.copy(gc1sb, psGC1)
        nc.sync.dma_start(out=d_ex_l[EX_C1:EX_G2].rearrange("(c p) -> c p", p=512),
                          in_=gc1sb[8:24, :])
        nc.sync.dma_start(out=d_ex_l[EX_G1:EX_ST].rearrange("(c p) -> c p", p=512),
                          in_=gc1sb[0:8, :])
        gp11l = dpool.tile([8, 512], f16, name="gp11l")
        nc.scalar.activation(out=gp11l, in_=gc1sb[0:8, :], func=AF.Identity,
                             bias=gb1_t[:, 0:1])

        # G2|C2: both exchanged (the full-image tail needs them)
        psGC2 = small_conv(wgc2_t, 17, h2p, "psGC2")
        gc2sb = dpool.tile([17, 512], f32, name="gc2sb")
        nc.scalar.copy(gc2sb, psGC2)
        nc.sync.dma_start(out=d_ex_l[EX_G2:EX_C2].rearrange("(c p) -> c p", p=512),
                          in_=gc2sb[0:16, :])
        nc.sync.dma_start(out=d_ex_l[EX_C2:EX_G1], in_=gc2sb[16:17, :])

        # ---------------- LN0 partial stats (local) --------
        bno = dpool.tile([128, 12, 6], f32)
        for kc in range(12):
            nc.vector.bn_stats(out=bno[:, kc, :], in_=xt[:, kc, :])
        mv0 = spool.tile([128, 2], f32, tag="mv0")
        nc.vector.bn_aggr(out=mv0, in_=bno)
        m20 = spool.tile([128, 1], f32, tag="m20")
        nc.vector.tensor_tensor(out=m20, in0=mv0[:, 0:1], in1=mv0[:, 0:1], op=ALU.mult)
        nc.vector.tensor_tensor(out=mv0[:, 1:2], in0=mv0[:, 1:2], in1=m20, op=ALU.add)
        psSt0 = SMALL([1, 2], "psSt0")
        nc.tensor.matmul(psSt0, ones32, mv0, start=True, stop=True)
        stsb0 = spool.tile([1, 2], f32, tag="stsb0")
        nc.scalar.copy(stsb0, psSt0)
        nc.sync.dma_start(out=d_ex_l[EX_ST:EX_N], in_=stsb0[0:1, 0:2])

        # ---------------- collectives on the gpsimd queue ----------------
        # dummy first: absorbs ncfw's boot + first-collective setup
        nc.gpsimd.collective_compute(
            "AllGather", ALU.bypass, replica_groups=PAIRS,
            ins=[warm_l[:]], outs=[warm_g[:]])
        nc.gpsimd.collective_compute(
            "AllGather", ALU.bypass, replica_groups=PAIRS,
            ins=[d_ex_l[:]], outs=[d_ex_g[:, :]])
        # readbacks (queue after the early collective)
        stg0 = spool.tile([16, 4], f32, tag="stg0")
        nc.gpsimd.dma_start(
            out=stg0.rearrange("p (h c) -> p h c", c=2),
            in_=d_ex_g[:, EX_ST:EX_N][None].to_broadcast([16, 2, 2]))
        g1f = dpool.tile([8, 2, 512], f32, name="g1f")
        nc.gpsimd.dma_start(
            out=g1f, in_=d_ex_g[:, EX_G1:EX_ST].rearrange("h (c p) -> c h p", p=512))
        c1f = dpool.tile([16, 2, 512], f32, name="c1f")
        nc.gpsimd.dma_start(
            out=c1f, in_=d_ex_g[:, EX_C1:EX_G2].rearrange("h (c p) -> c h p", p=512))
        g2f = dpool.tile([16, 2, 512], f32, name="g2f")
        nc.gpsimd.dma_start(
            out=g2f, in_=d_ex_g[:, EX_G2:EX_C2].rearrange("h (c p) -> c h p", p=512))
        c2f = dpool.tile([1, 2, 512], f32, name="c2f")
        nc.gpsimd.dma_start(
            out=c2f, in_=d_ex_g[:, EX_C2:EX_G1].rearrange("h (c p) -> c h p", p=512))

        # ---------------- conv_g + A0 ----------------
        # the LN0 r0/nm0 chain is emitted into the DVE queue after kc==9's
        # multiply so it runs while the PE finishes the last kc blocks
        # (its input stg0 arrives from the early exchange mid-conv_g)
        psA0 = ps.tile([8, 512], f32, tag="ps_a0", bufs=1, name="psA0")
        r0 = nm0 = bias0 = u0l = None
        for kc in range(12):
            psg = MAIN([128, 512], f"psg{kc}")
            for t, (dy, dx) in enumerate(TAPS):
                nc.tensor.matmul(psg, wg_t[:, kc, t, :],
                                 h0p[:, dy:dy + 16, dx:dx + 32],
                                 start=(t == 0), stop=(t == 8))
            gp1 = dpool.tile([128, 512], f16, tag="gp1", bufs=2, name=f"gp1_{kc}")
            nc.scalar.activation(out=gp1, in_=psg, func=AF.Identity,
                                 bias=gb0_t[:, kc:kc + 1])
            xg = dpool.tile([128, 512], f16, tag="xg", bufs=2, name=f"xg_{kc}")
            nc.vector.tensor_tensor(out=xg, in0=xt[:, kc, :],
                                    in1=gp1, op=ALU.mult)
            nc.tensor.matmul(psA0, w0t_t[:, kc, :], xg,
                             start=(kc == 0), stop=(kc == 11))
            if kc == 10:
                gp11f = dpool.tile([8, 2, 512], f16, name="gp11f")
                nc.scalar.activation(out=gp11f, in_=g1f, func=AF.Identity,
                                     bias=gb1_t[:, 0:1])
            if kc == 9:
                r0, nm0 = _stats_chain(nc, spool, stg0, 256.0, 16, 0, pair=True)
                bias0 = spool.tile([8, 1], f32, tag="bias0")
                nc.vector.tensor_scalar(out=bias0, in0=b0cb,
                                        scalar1=nm0[0:8, 0:1],
                                        scalar2=b0fb[:, 0:1],
                                        op0=ALU.mult, op1=ALU.add)
                u0l = dpool.tile([8, 512], f32, tag="sU", name="u0l")
                nc.vector.affine_then_add(out=u0l, in0=bc0sb[0:8, :], in1=c0sb,
                                          scale=nm0[0:8, 0:1], bias=0.0)

        # ------- local finish of layer 0 for our own half ----------
        z0l = dpool.tile([8, 512], f32, tag="sZ", name="z0l")
        nc.vector.affine_then_add(out=z0l, in0=psA0, in1=u0l,
                                  scale=r0[0:8, 0:1], bias=0.0)
        te0 = dpool.tile([8, 512], f32, tag="sT", name="te0")
        nc.scalar.activation(out=te0, in_=z0l, func=AF.Exp, bias=bias0[:, 0:1])
        sq1 = spool.tile([8, 2], f32, tag="sq1")
        out0 = dpool.tile([8, 512], f32, tag="sO", name="out0")
        nc.scalar.activation(out=out0, in_=te0, func=AF.Ln, bias=1.0,
                             accum_out=sq1[:, 0:1])
        og1 = dpool.tile([8, 512], f16, tag="sG", name="og1")
        nc.vector.tensor_tensor(out=og1, in0=out0, in1=gp11l, op=ALU.mult)
        scr0 = dpool.tile([8, 512], f32, tag="sS", name="scr0")
        nc.vector.affine_mul_reduce(out=scr0, accum_out=sq1[:, 1:2],
                                    in0=out0, in1=out0,
                                    scale=ones32[0:8, 0:1], bias=0.0)
        nc.sync.dma_start(out=d_lt_l[LT_OG:LT_SQ].rearrange("(c p) -> c p", p=512),
                          in_=og1)
        nc.sync.dma_start(out=d_lt_l[LT_SQ:LT_N], in_=sq1[:, :].bitcast(f16))
        # B1 full image = W1^T gp11f (PE is free once conv_g ends)
        b1sb = dpool.tile([16, 2, 512], f32, name="b1sb")
        for hh in range(2):
            psB1f = TAIL([16, 512], f"psB1f{hh}")
            nc.tensor.matmul(psB1f, w1t_t, gp11f[:, hh, :], start=True, stop=True)
            nc.scalar.copy(b1sb[:, hh, :], psB1f)

        # late exchange
        nc.gpsimd.collective_compute(
            "AllGather", ALU.bypass, replica_groups=PAIRS,
            ins=[d_lt_l[:]], outs=[d_lt_g[:, :]])
        # sq first -- it gates the LN1 stats chain; halves stacked on
        # partitions so one all-ones matmul does reduce+broadcast
        sq1g = spool.tile([16, 4], f16, tag="sq1g")
        nc.gpsimd.dma_start(
            out=sq1g[0:8, :],
            in_=d_lt_g[0, LT_SQ:LT_N].rearrange("(c q) -> c q", q=4))
        nc.gpsimd.dma_start(
            out=sq1g[8:16, :],
            in_=d_lt_g[1, LT_SQ:LT_N].rearrange("(c q) -> c q", q=4))
        og1f = dpool.tile([8, 2, 512], f16, name="og1f")
        nc.gpsimd.dma_start(
            out=og1f, in_=d_lt_g[:, LT_OG:LT_SQ].rearrange("h (c p) -> c h p", p=512))

        # ---- full-image precompute that hides under conv_g / late op ----
        gp12f = dpool.tile([16, 2, 512], f16, name="gp12f")
        nc.scalar.activation(out=gp12f, in_=g2f, func=AF.Identity,
                             bias=gb2_t[:, 0:1])
        b2sb = dpool.tile([1, 2, 512], f32, name="b2sb")
        for hh in range(2):
            psB2 = SMALL([1, 512], f"psB2{hh}")
            nc.tensor.matmul(psB2, w2t_t, gp12f[:, hh, :], start=True, stop=True)
            nc.scalar.copy(b2sb[:, hh, :], psB2)

        # ---------------- post-exchange tail (full image) ----------------
        # LN1 totals+broadcast in one all-ones matmul: [16,4] rows all equal
        # (S_h0, Q_h0, S_h1, Q_h1) summed over the 8 channels
        psStB1 = SMALL([16, 2], "psStB1")
        nc.tensor.matmul(psStB1, allon[0:16, :], sq1g[:, :].bitcast(f32),
                         start=True, stop=True)
        psA1f = [TAIL([16, 512], f"psA1f{hh}") for hh in range(2)]
        for hh in range(2):
            nc.tensor.matmul(psA1f[hh], w1t_t, og1f[:, hh, :],
                             start=True, stop=True)
        r1, nm1 = _stats_chain(nc, spool, psStB1, 8192.0, 16, 1, n_iter=1)
        # layer 1+2 run per-half so the DVE works on one half while the ACT
        # softplus runs on the other; LN2 sums land in per-half slots of
        # sq2 [16,4] and the all-ones matmul reduces across both
        u1 = dpool.tile([16, 2, 512], f32, tag="sU16", name="u1")
        z1 = dpool.tile([16, 2, 512], f32, tag="sZ16", name="z1")
        te1 = dpool.tile([16, 2, 512], f32, tag="sT", name="te1")
        sq2 = spool.tile([16, 4], f32, tag="sq2")
        out1 = dpool.tile([16, 2, 512], f32, tag="sO16", name="out1")
        scr1 = dpool.tile([16, 2, 512], f32, tag="sS16", name="scr1")
        og2 = dpool.tile([16, 2, 512], f16, tag="sG", name="og2")
        psA2 = [TAIL([1, 512], f"psA2{hh}") for hh in range(2)]
        for hh in range(2):
            nc.vector.affine_then_add(out=u1[:, hh, :], in0=b1sb[:, hh, :],
                                      in1=c1f[:, hh, :],
                                      scale=nm1[:, 0:1], bias=0.0)
            nc.vector.affine_then_add(out=z1[:, hh, :], in0=psA1f[hh],
                                      in1=u1[:, hh, :],
                                      scale=r1[:, 0:1], bias=0.0)
            nc.scalar.activation(out=te1[:, hh, :], in_=z1[:, hh, :],
                                 func=AF.Exp, bias=b1fb[:, 0:1])
            nc.scalar.activation(out=out1[:, hh, :], in_=te1[:, hh, :],
                                 func=AF.Ln, bias=1.0,
                                 accum_out=sq2[:, 2 * hh:2 * hh + 1])
            nc.vector.affine_mul_reduce(out=scr1[:, hh, :],
                                        accum_out=sq2[:, 2 * hh + 1:2 * hh + 2],
                                        in0=out1[:, hh, :], in1=out1[:, hh, :],
                                        scale=ones32[0:16, 0:1], bias=0.0)
            nc.vector.tensor_tensor(out=og2[:, hh, :], in0=out1[:, hh, :],
                                    in1=gp12f[:, hh, :], op=ALU.mult)
            nc.tensor.matmul(psA2[hh], w2t_t, og2[:, hh, :], start=True, stop=True)
        psSt2 = SMALL([1, 4], "psSt2")
        nc.tensor.matmul(psSt2, ones32[0:16, :], sq2, start=True, stop=True)
        st2sb = spool.tile([1, 4], f32, tag="st2sb")
        nc.scalar.copy(st2sb, psSt2)
        r2, nm2 = _stats_chain(nc, spool, st2sb, 16384.0, 1, 2, n_iter=1,
                               pair=True)

        u2 = dpool.tile([1, 2, 512], f32, tag="sU16", name="u2")
        z2 = dpool.tile([1, 2, 512], f32, tag="sZ", name="z2")
        te2 = dpool.tile([1, 2, 512], f32, tag="sT", name="te2")
        final = dpool.tile([1, 2, 512], f32, tag="sO", name="final")
        for hh in range(2):
            nc.vector.affine_then_add(out=u2[:, hh, :], in0=b2sb[:, hh, :],
                                      in1=c2f[:, hh, :],
                                      scale=nm2[:, 0:1], bias=0.0)
            nc.vector.affine_then_add(out=z2[:, hh, :], in0=psA2[hh],
                                      in1=u2[:, hh, :],
                                      scale=r2[:, 0:1], bias=0.0)
            nc.scalar.activation(out=te2[:, hh, :], in_=z2[:, hh, :],
                                 func=AF.Exp, bias=b2fb[:, 0:1])
            nc.scalar.activation(out=final[:, hh, :], in_=te2[:, hh, :],
                                 func=AF.Ln, bias=1.0)
            nc.sync.dma_start(out=d_out[512 * hh:512 * hh + 512],
                              in_=final[0:1, hh, :])

    nc.compile()
    return nc


def _host_prep(inputs):
    """Build per-core in_maps (host work: slicing, layout, small weight folds)."""
    x_main = np.asarray(inputs["x_main"], np.float32)
    f_sem = np.asarray(inputs["f_sem"], np.float32)
    seg = np.asarray(inputs["seg_mask"])

    ws_stack = np.stack([inputs["s0_ws"], inputs["s1_ws"], inputs["s2_ws"]])
    # WST[i_lo, mc, cv*1152 + t*128 + o] = ws_cv[o, mc*128+i_lo, ky, kx]
    arr = np.asarray(ws_stack, np.float32).reshape(3, 128, 3, 128, 3, 3)
    WST = np.ascontiguousarray(
        arr.transpose(3, 2, 0, 4, 5, 1).reshape(128, 3, 3456)).astype(np.float16)

    wg0 = np.asarray(inputs["s0_wg"], np.float32)          # [1536, 128, 3, 3]
    WG = np.ascontiguousarray(
        wg0.reshape(12, 128, 128, 3, 3).transpose(2, 0, 3, 4, 1)
        .reshape(128, 12, 9, 128)).astype(np.float16)
    W0T = np.ascontiguousarray(np.asarray(inputs["conv0_w"], np.float32).T
                               .reshape(12, 128, 8).transpose(1, 0, 2)).astype(np.float16)

    c0w = np.asarray(inputs["conv0_w"], np.float64)
    c1w = np.asarray(inputs["conv1_w"], np.float64)
    c2w = np.asarray(inputs["conv2_w"], np.float64)
    foldg0 = np.einsum("oc,cikl->oikl", c0w, np.asarray(inputs["s0_wg"], np.float64))
    wf0 = np.einsum("oc,cikl->oikl", c0w, np.asarray(inputs["s0_wb"], np.float64))
    wf1 = np.einsum("oc,cikl->oikl", c1w, np.asarray(inputs["s1_wb"], np.float64))
    wf2 = np.einsum("oc,cikl->oikl", c2w, np.asarray(inputs["s2_wb"], np.float64))

    def lhsT(w):   # [O, 128, 3, 3] -> [128 j, 9 t, O]
        return np.ascontiguousarray(w.transpose(1, 2, 3, 0).reshape(128, 9, w.shape[0]))

    WBC0 = np.concatenate([lhsT(foldg0), lhsT(wf0)], axis=2).astype(np.float16)
    WGC1 = np.concatenate([lhsT(np.asarray(inputs["s1_wg"], np.float64)),
                           lhsT(wf1)], axis=2).astype(np.float16)
    WGC2 = np.concatenate([lhsT(np.asarray(inputs["s2_wg"], np.float64)),
                           lhsT(wf2)], axis=2).astype(np.float16)
    W12 = np.zeros((16, 17), np.float16)
    W12[0:8, 0:16] = np.asarray(inputs["conv1_w"], np.float32).T.astype(np.float16)
    W12[0:16, 16] = np.asarray(inputs["conv2_w"], np.float32).T[:, 0].astype(np.float16)
    BS = np.ascontiguousarray(np.stack([inputs["s0_bs"], inputs["s1_bs"],
                                        inputs["s2_bs"]]).T).astype(np.float32)
    GB0 = np.ascontiguousarray((1.0 + np.asarray(inputs["s0_bg"], np.float32))
                               .reshape(12, 128).T).astype(np.float32)
    # sc2 cols: gb2(16) | b1f(16) | gb1(8) | b0f(8) | b0c(8) | b2f(1)
    SC2 = np.zeros((16, 6), np.float64)
    SC2[:, 0] = 1.0 + np.asarray(inputs["s2_bg"], np.float64)
    SC2[:, 1] = (np.asarray(inputs["b1"], np.float64)
                 + c1w @ np.asarray(inputs["s1_bb"], np.float64))
    SC2[0:8, 2] = 1.0 + np.asarray(inputs["s1_bg"], np.float64)
    SC2[0:8, 3] = (np.asarray(inputs["b0"], np.float64)
                   + c0w @ np.asarray(inputs["s0_bb"], np.float64))
    SC2[0:8, 4] = c0w @ (1.0 + np.asarray(inputs["s0_bg"], np.float64))
    SC2[0, 5] = (np.asarray(inputs["b2"], np.float64)
                 + c2w @ np.asarray(inputs["s2_bb"], np.float64))[0]
    SC2 = SC2.astype(np.float32)

    shared = dict(wst=WST, wg=WG, w0t=W0T, wbc0=WBC0, wgc1=WGC1, wgc2=WGC2,
                  w12=W12, bs=BS, gb0=GB0, sc2=SC2)

    in_maps = []
    for core in range(8):
        k, h = core // 2, core % 2
        r0 = HROWS * h
        X = np.ascontiguousarray(
            x_main[k, :, r0:r0 + HROWS, :].reshape(12, 128, 512).transpose(1, 0, 2)
        ).astype(np.float16)
        FT = np.ascontiguousarray(
            f_sem[k].reshape(384, NPOS).T.reshape(8, 128, 384).transpose(1, 0, 2)
        ).astype(np.float16)
        ids_flat = seg[k, ::14, ::14].astype(np.float32).reshape(NPOS)
        IDS = np.ascontiguousarray(ids_flat.reshape(8, 128).T)
        rows = np.arange(r0 - 2, r0 + HROWS + 2)          # 20 Gr rows
        valid = (rows >= 0) & (rows < Hp)
        rcl = np.clip(rows, 0, Hp - 1)
        cid = np.empty((GRR, Wp, 4), np.float32)
        cols = np.arange(Wp)
        for t, (dy, dx) in enumerate([(0, 0), (0, 1), (1, 0), (1, 1)]):
            v = seg[k][np.ix_(14 * rcl + 6 + dy, 14 * cols + 6 + dx)].astype(np.float32)
            v[~valid, :] = -1.0
            cid[:, :, t] = v
        CID = np.ascontiguousarray(cid.reshape(5, 128, 4).transpose(1, 0, 2))
        hrows = np.arange(r0 - 1, r0 + HROWS + 1)
        HM = ((hrows >= 0) & (hrows < Hp)).astype(np.float16)
        in_maps.append(dict(shared, x=X, ft=FT, ids=IDS, cid=CID, hmask=HM))
    return in_maps


def kernel(**inputs):
    global _BUILT, LAST_RESULTS
    if _BUILT is None:
        _BUILT = _build_nc()
    nc = _BUILT
    in_maps = _host_prep(inputs)
    trace = bool(os.environ.get("BASS_TRACE"))
    res = run_bass_kernel_spmd(nc, in_maps, list(range(8)), trace=trace)
    LAST_RESULTS = res
    out = np.empty((B, 1, Hp, Wp), np.float32)
    for k in range(B):
        full = res.results[2 * k]["out_full"].reshape(2, HROWS, Wp)
        out[k, 0, 0:HROWS, :] = full[0]
        out[k, 0, HROWS:, :] = full[1]
    return out
